# revision 12
# baseline (speedup 1.0000x reference)
"""Contrastive loss (SimCLR-style) on 8 Trainium2 NeuronCores.

Full inputs in, full output out.  Host pre-normalizes feats (f32), takes
the positive-pair cosines on the host (8192x128 dot products - trivial
next to the 256MB label argmax already done there), and ships each core
a rolled, transposed bf16 copy nfT = roll(nf).T of the 5120 columns the
core actually touches (1.25MB).  The device then only does the heavy
part: the N/8 x 5N/8 block of similarities, exp, and row/column sums.

Symmetry split: exp(cos/T) is symmetric, so core x only computes its
1024 rows against local column blocks 0..4 (cols 0..5119).  Row sums
over the remaining column blocks 5..7 are recovered from *column* sums
of blocks (x, x+1..x+3), which other cores' rows need by symmetry.
Per M-tile the 5120 columns run as PSUM spans of 2048/2048/1024 so the
ACT exp (the bottleneck engine) pays its 352-cycle ramp 24x, not 40x.
Column sums accumulate on the otherwise-idle DVE in bf16; the final
partition reduction is 24 skinny stationary=acc matmuls that write a
transposed [128, 24] tile so the output DMA uses all 128 partitions
instead of a slow single-partition 12KB line.  Host: assemble S from
row partials + shipped column sums, logsumexp, mean.
"""

from contextlib import ExitStack

import numpy as np

N, D, NCORES = 8192, 128, 8
BLK = N // NCORES            # 1024 rows per core
TPB = BLK // 128             # 8 M-tiles (of 128 rows) per core
TEMP = 0.07
EPS = 1e-8
MASK_SUB = 30.0              # cos - 30 -> exp((cos-30)/T) == 0 in fp32
QCOLS = 1024                 # one column block
MMCHUNK = 512                # matmul moving-operand columns
NQ = 5                       # direct column blocks per core (cols 0..5119)
CSBLKS = 3                   # column-sum blocks (local col blocks 1..3)
NCOLS = NQ * QCOLS           # 5120 columns shipped per core
CSOUT = CSBLKS * TPB + TPB - 1 + 4   # shipped colsum chunks (inter|diag|diff4)

_CACHE = {}
LAST_RESULT = None


def _emit(tc, nfT_d, masks_d, s_out, cs_out, rep=0):
    import concourse.mybir as mybir

    nc = tc.nc
    f32 = mybir.dt.float32
    bf16 = mybir.dt.bfloat16
    AF = mybir.ActivationFunctionType
    AX = mybir.AxisListType.X

    with ExitStack() as ctx:
        singles = ctx.enter_context(tc.tile_pool(name=f"singles{rep}", bufs=1))

        nfT = singles.tile([128, NCOLS], bf16, tag="nfT")    # normalized X^T
        masks = singles.tile([128, 256], bf16, tag="masks")  # [ident | -30*ident]
        ones = singles.tile([128, 512], bf16, tag="ones")
        parts = singles.tile([128, TPB * 4], f32, tag="parts")
        sv = singles.tile([128, TPB], f32, tag="sv")
        acc = singles.tile([128, CSBLKS * QCOLS], bf16, tag="acc")
        acc2 = singles.tile([128, (TPB - 1) * 128], bf16, tag="acc2")
        acc3 = singles.tile([128, 512], bf16, tag="acc3")
        csg = singles.tile([128, CSBLKS * TPB + TPB - 1 + 4], f32, tag="csg")
        wact = singles.tile([128, 8], f32, tag="wact")

        # ---- ACT warmup, overlapped with the input DMA window ----
        # The first ACTIVATE pays the ~2.7us exp table load; trigger it at
        # t=0 on a tiny memset tile so the load hides under the nfT DMA.
        # (No PE warmup: its PSUM slot reuse makes the first real matmul
        # wait for every dummy one, which costs more than the cold clock.)
        nc.vector.memset(wact[:], 0.0)
        nc.scalar.activation(wact[:], wact[:], AF.Exp)
        nc.vector.memset(ones[:], 1.0)

        # ---- loads (chunk0 gates the first span; masks right behind) ----
        nc.sync.dma_start(out=nfT[:, 0:QCOLS], in_=nfT_d[:, 0:QCOLS])
        nc.sync.dma_start(out=masks[:], in_=masks_d)
        for a in range(QCOLS, NCOLS, QCOLS):
            nc.sync.dma_start(out=nfT[:, a:a + QCOLS], in_=nfT_d[:, a:a + QCOLS])

        # ---- similarity spans + exp row-sums + column sums ----
        # The diagonal block is symmetric, so M-tile m only computes cols
        # >= m*128 of it (trapezoid span); the missing lower-triangle row
        # contributions are recovered from colsums of earlier tiles' exp
        # (chunks accumulated into acc2) and folded into sv on-device.
        # m=0 runs 1024-wide spans so the first exp is gated by only two
        # matmuls on the freshly-DMAed chunk0, not four.
        SPANS0 = ((0, 1024, 0), (1024, 1024, 1), (2048, 2048, 2), (4096, 512, 3))
        with (
            tc.tile_pool(name=f"mpsum{rep}", bufs=2, space="PSUM") as mpsum,
            tc.tile_pool(name=f"escratch{rep}", bufs=3) as esp,
            tc.high_priority(),
        ):
            for m in range(TPB):
                lhsT = nfT[:, m * 128:(m + 1) * 128]
                if m == 0:
                    spans = SPANS0
                else:
                    w2 = 512 if m < TPB // 2 else 1024
                    spans = ((m * 128, 2048 - m * 128, 0), (2048, 2048, 1),
                             (4096, w2, 2))
                for c0, w, slot in spans:
                    pt = mpsum.tile([128, 2048], f32, tag="mp")
                    b = c0
                    while b < c0 + w:
                        cw = min(MMCHUNK, c0 + w - b)
                        nc.tensor.matmul(
                            pt[:, b - c0:b - c0 + cw],
                            lhsT, nfT[:, b:b + cw],
                            start=True, stop=True,
                        )
                        b += cw
                    if c0 == m * 128:
                        # self column of local row m*128+p is m*128+p (rolled
                        # input): accumulate (-30I)^T @ I onto the diagonal
                        # sub-block on the PE so exp never waits on another
                        # engine.
                        nc.tensor.matmul(
                            pt[:, 0:128],
                            masks[:, 128:256], masks[:, 0:128],
                            start=False, stop=True, skip_group_check=True,
                        )
                    e = esp.tile([128, 2048], bf16, tag="e")
                    nc.scalar.activation(
                        e[:, :w], pt[:, :w], AF.Exp, scale=1.0 / TEMP,
                        accum_out=parts[:, m * 4 + slot:m * 4 + slot + 1],
                    )
                    # column-sum accumulation (cols 1024..4095) on DVE
                    lo = max(c0, QCOLS)
                    hi = min(c0 + w, (CSBLKS + 1) * QCOLS)
                    if lo < hi:
                        a = acc[:, lo - QCOLS:hi - QCOLS]
                        eslice = e[:, lo - c0:hi - c0]
                        if m == 0:
                            nc.vector.tensor_copy(a, eslice)
                        else:
                            nc.vector.tensor_add(a, a, eslice)
                    # diff-4 left half, bottom-half rows: colsums feed the
                    # partner core's top-half rows (via the host)
                    if c0 == 4096 and m >= TPB // 2:
                        a3 = acc3[:]
                        eslice = e[:, 0:512]
                        if m == TPB // 2:
                            nc.vector.tensor_copy(a3, eslice)
                        else:
                            nc.vector.tensor_add(a3, a3, eslice)
                    # diag-block intra colsums: chunks c>m of this tile's
                    # trapezoid feed later tiles' row sums
                    if c0 < QCOLS:
                        for c in range(max(m + 1, c0 // 128), min((c0 + w) // 128, TPB)):
                            a2 = acc2[:, (c - 1) * 128:c * 128]
                            eslice = e[:, c * 128 - c0:(c + 1) * 128 - c0]
                            if m == 0:
                                nc.vector.tensor_copy(a2, eslice)
                            else:
                                nc.vector.tensor_add(a2, a2, eslice)
                nsl = 4 if m == 0 else 3
                nc.vector.reduce_sum(out=sv[:, m:m + 1],
                                     in_=parts[:, m * 4:m * 4 + nsl], axis=AX)

        # ---- tail: transpose-reduce column sums ----
        NK = CSBLKS * TPB
        NK2 = NK + TPB - 1
        with tc.tile_pool(name=f"tpsum{rep}", bufs=1, space="PSUM") as tpsum:
            cspT = tpsum.tile([128, NK2 + 4], f32, tag="cspT")
            for k in range(NK):
                # cspT[c, k] = sum_p acc[p, 128k + c]  (all output cols equal)
                nc.tensor.matmul(cspT[:, k:k + 1],
                                 acc[:, k * 128:(k + 1) * 128], ones[:, 0:1],
                                 start=True, stop=True)
            for c in range(1, TPB):
                nc.tensor.matmul(cspT[:, NK + c - 1:NK + c],
                                 acc2[:, (c - 1) * 128:c * 128], ones[:, 0:1],
                                 start=True, stop=True)
            for k in range(4):
                nc.tensor.matmul(cspT[:, NK2 + k:NK2 + k + 1],
                                 acc3[:, k * 128:(k + 1) * 128], ones[:, 0:1],
                                 start=True, stop=True)
            nc.vector.tensor_copy(csg[:], cspT[:])
        nc.sync.dma_start(out=s_out, in_=sv[:])
        nc.sync.dma_start(out=cs_out, in_=csg[:])


def _build_nc(repeats=1):
    import concourse.tile as tile
    import concourse.mybir as mybir
    from concourse import bacc

    f32 = mybir.dt.float32
    bf16 = mybir.dt.bfloat16
    nc = bacc.Bacc(
        "TRN2", target_bir_lowering=False, debug=False,
        enable_asserts=False, num_devices=NCORES,
    )
    nfT_h = nc.dram_tensor("nfT", [128, NCOLS], bf16, kind="ExternalInput")
    mk_h = nc.dram_tensor("masks", [128, 256], bf16, kind="ExternalInput")
    s_h = nc.dram_tensor("s_out", [128, TPB], f32, kind="ExternalOutput")
    c_h = nc.dram_tensor("cs_out", [128, CSOUT], f32, kind="ExternalOutput")

    with tile.TileContext(nc, trace_sim=False) as tc:
        for rep in range(repeats):
            _emit(tc, nfT_h.ap(), mk_h.ap(), s_h.ap(), c_h.ap(), rep=rep)
    nc.compile()
    return nc


def get_nc(repeats=1):
    key = ("nc", repeats)
    if key not in _CACHE:
        _CACHE[key] = _build_nc(repeats)
    return _CACHE[key]


def _host_prep(feats, label):
    """Normalize on host, build per-core rolled nfT (bf16) and host pos."""
    import ml_dtypes

    feats = np.asarray(feats, dtype=np.float32)
    label = np.asarray(label)
    norms = np.sqrt((feats.astype(np.float64) ** 2).sum(axis=1))
    nf = feats / np.maximum(norms, EPS)[:, None].astype(np.float32)
    pos_idx = np.argmax(label, axis=1)
    pos = np.einsum("ij,ij->i", nf.astype(np.float64), nf[pos_idx].astype(np.float64))
    nfT_full = np.ascontiguousarray(nf.T.astype(ml_dtypes.bfloat16))  # [128, N]
    nfT2 = np.concatenate([nfT_full, nfT_full[:, :NCOLS]], axis=1)
    masks = np.concatenate([np.eye(128), -MASK_SUB * np.eye(128)],
                           axis=1).astype(ml_dtypes.bfloat16)
    in_maps = []
    for c in range(NCORES):
        in_maps.append({
            "nfT": np.ascontiguousarray(nfT2[:, c * BLK:c * BLK + NCOLS]),
            "masks": masks,
        })
    return in_maps, pos


def make_in_maps(feats, label):
    in_maps, _ = _host_prep(feats, label)
    return in_maps


def finish(results, pos):
    """Host epilogue: assemble full row sums from direct row partials and
    symmetric column partials, then logsumexp and mean."""
    NK = CSBLKS * TPB
    NK2 = NK + TPB - 1
    S = np.zeros(N, dtype=np.float64)
    for x in range(NCORES):
        sv = results[x]["s_out"].astype(np.float64)       # [128, TPB]
        S[x * BLK:(x + 1) * BLK] += sv.T.reshape(-1)      # local rows in order
        csg = results[x]["cs_out"].astype(np.float64)     # [128, CSOUT]
        # cols 0..NK: colsums of local column 128k+c (k = 8*(q-1)+sub)
        cs = csg[:, :NK].T.reshape(CSBLKS, BLK)
        for k in range(1, CSBLKS + 1):
            tgt = ((x + k) % NCORES) * BLK                # rows of block x+k
            S[tgt:tgt + BLK] += cs[k - 1]
        # cols NK..NK2: diag-block lower-triangle sums for own rows 128..1023
        S[x * BLK + 128:(x + 1) * BLK] += csg[:, NK:NK2].T.reshape(-1)
        # cols NK2..: diff-4 bottom-left quadrant sums for partner rows 0..511
        tgt = ((x + 4) % NCORES) * BLK
        S[tgt:tgt + 512] += csg[:, NK2:NK2 + 4].T.reshape(-1)
    lse = np.log(S)
    loss = (lse - pos / TEMP).mean()
    return np.array(loss, dtype=np.float32)


def kernel(feats, label, _trace=False, _repeats=1):
    global LAST_RESULT
    from concourse.bass_utils import run_bass_kernel_spmd

    nc = get_nc(_repeats)
    in_maps, pos = _host_prep(feats, label)
    res = run_bass_kernel_spmd(nc, in_maps, list(range(NCORES)), trace=_trace)
    LAST_RESULT = res
    return finish(res.results, pos)


# revision 13
# speedup vs baseline: 1.0275x; 1.0275x over previous
"""Contrastive loss (SimCLR-style) on 8 Trainium2 NeuronCores.

Full inputs in, full output out.  Host pre-normalizes feats (f32), takes
the positive-pair cosines on the host (8192x128 dot products - trivial
next to the 256MB label argmax already done there), and ships each core
a rolled, transposed bf16 copy nfT = roll(nf).T of the 5120 columns the
core actually touches (1.25MB).  The device then only does the heavy
part: the N/8 x 5N/8 block of similarities, exp, and row/column sums.

Symmetry split, three levels (exp(cos/T) is symmetric):
 - inter-block: core x computes column blocks x..x+4 directly; row sums
   over blocks x+5..x+7 are recovered from column sums of (x, x+1..x+3).
 - diagonal block: M-tile m computes only cols >= m*128 (trapezoid); the
   lower triangle comes back as column sums of earlier tiles (acc2).
 - diff-4 block (computed by both x and x+4): each core does the left
   512 cols for its top-half rows plus full width for its bottom half;
   bottom-left-quadrant column sums are shipped to the partner (acc3).
Per M-tile the columns run as PSUM spans up to 2048 wide so the ACT exp
(the bottleneck engine - exp only exists there, (N+352)cyc @1.2GHz)
amortizes its per-instruction ramp; an exp-table warmup at t=0 hides
the ~2.7us table load under the input DMA.  The self-mask is a PE
accumulate-matmul (-30I)^T @ I so exp never waits on another engine.
Column sums accumulate on the otherwise-idle DVE in bf16; the final
partition reduction is 35 skinny stationary=acc matmuls that write a
transposed [128, 35] tile so the output DMA uses all 128 partitions
instead of a slow single-partition line.  Host: assemble S from row
partials + shipped column sums, logsumexp, mean over host-side pos.
"""

from contextlib import ExitStack

import numpy as np

N, D, NCORES = 8192, 128, 8
BLK = N // NCORES            # 1024 rows per core
TPB = BLK // 128             # 8 M-tiles (of 128 rows) per core
TEMP = 0.07
EPS = 1e-8
MASK_SUB = 30.0              # cos - 30 -> exp((cos-30)/T) == 0 in fp32
QCOLS = 1024                 # one column block
MMCHUNK = 512                # matmul moving-operand columns
NQ = 5                       # direct column blocks per core (cols 0..5119)
CSBLKS = 3                   # column-sum blocks (local col blocks 1..3)
NCOLS = NQ * QCOLS           # 5120 columns shipped per core
CSOUT = CSBLKS * TPB + TPB - 1 + 4   # shipped colsum chunks (inter|diag|diff4)

_CACHE = {}
LAST_RESULT = None


def _emit(tc, nfT_d, masks_d, s_out, cs_out, rep=0):
    import concourse.mybir as mybir

    nc = tc.nc
    f32 = mybir.dt.float32
    bf16 = mybir.dt.bfloat16
    AF = mybir.ActivationFunctionType
    AX = mybir.AxisListType.X

    with ExitStack() as ctx:
        singles = ctx.enter_context(tc.tile_pool(name=f"singles{rep}", bufs=1))

        nfT = singles.tile([128, NCOLS], bf16, tag="nfT")    # normalized X^T
        masks = singles.tile([128, 256], bf16, tag="masks")  # [ident | -30*ident]
        ones = singles.tile([128, 512], bf16, tag="ones")
        parts = singles.tile([128, TPB * 4], f32, tag="parts")
        sv = singles.tile([128, TPB], f32, tag="sv")
        acc = singles.tile([128, CSBLKS * QCOLS], bf16, tag="acc")
        acc2 = singles.tile([128, (TPB - 1) * 128], bf16, tag="acc2")
        acc3 = singles.tile([128, 512], bf16, tag="acc3")
        csg = singles.tile([128, CSBLKS * TPB + TPB - 1 + 4], f32, tag="csg")
        wact = singles.tile([128, 8], f32, tag="wact")

        # ---- ACT warmup, overlapped with the input DMA window ----
        # The first ACTIVATE pays the ~2.7us exp table load; trigger it at
        # t=0 on a tiny memset tile so the load hides under the nfT DMA.
        # (No PE warmup: its PSUM slot reuse makes the first real matmul
        # wait for every dummy one, which costs more than the cold clock.)
        nc.vector.memset(wact[:], 0.0)
        nc.scalar.activation(wact[:], wact[:], AF.Exp)
        nc.vector.memset(ones[:], 1.0)

        # ---- loads (chunk0 gates the first span; masks right behind) ----
        nc.sync.dma_start(out=nfT[:, 0:QCOLS], in_=nfT_d[:, 0:QCOLS])
        nc.sync.dma_start(out=masks[:], in_=masks_d)
        for a in range(QCOLS, NCOLS, QCOLS):
            nc.sync.dma_start(out=nfT[:, a:a + QCOLS], in_=nfT_d[:, a:a + QCOLS])

        # ---- similarity spans + exp row-sums + column sums ----
        # The diagonal block is symmetric, so M-tile m only computes cols
        # >= m*128 of it (trapezoid span); the missing lower-triangle row
        # contributions are recovered from colsums of earlier tiles' exp
        # (chunks accumulated into acc2) and folded into sv on-device.
        # m=0 runs 1024-wide spans so the first exp is gated by only two
        # matmuls on the freshly-DMAed chunk0, not four.
        SPANS0 = ((0, 1024, 0), (1024, 1024, 1), (2048, 2048, 2), (4096, 512, 3))
        with (
            tc.tile_pool(name=f"mpsum{rep}", bufs=2, space="PSUM") as mpsum,
            tc.tile_pool(name=f"escratch{rep}", bufs=3) as esp,
            tc.high_priority(),
        ):
            for m in range(TPB):
                lhsT = nfT[:, m * 128:(m + 1) * 128]
                if m == 0:
                    spans = SPANS0
                else:
                    w2 = 512 if m < TPB // 2 else 1024
                    spans = ((m * 128, 2048 - m * 128, 0), (2048, 2048, 1),
                             (4096, w2, 2))
                for c0, w, slot in spans:
                    pt = mpsum.tile([128, 2048], f32, tag="mp")
                    b = c0
                    while b < c0 + w:
                        cw = min(MMCHUNK, c0 + w - b)
                        nc.tensor.matmul(
                            pt[:, b - c0:b - c0 + cw],
                            lhsT, nfT[:, b:b + cw],
                            start=True, stop=True,
                        )
                        b += cw
                    if c0 == m * 128:
                        # self column of local row m*128+p is m*128+p (rolled
                        # input): accumulate (-30I)^T @ I onto the diagonal
                        # sub-block on the PE so exp never waits on another
                        # engine.
                        nc.tensor.matmul(
                            pt[:, 0:128],
                            masks[:, 128:256], masks[:, 0:128],
                            start=False, stop=True, skip_group_check=True,
                        )
                    e = esp.tile([128, 2048], bf16, tag="e")
                    nc.scalar.activation(
                        e[:, :w], pt[:, :w], AF.Exp, scale=1.0 / TEMP,
                        accum_out=parts[:, m * 4 + slot:m * 4 + slot + 1],
                    )
                    # column-sum accumulation (cols 1024..4095) on DVE
                    lo = max(c0, QCOLS)
                    hi = min(c0 + w, (CSBLKS + 1) * QCOLS)
                    if lo < hi:
                        a = acc[:, lo - QCOLS:hi - QCOLS]
                        eslice = e[:, lo - c0:hi - c0]
                        if m == 0:
                            nc.vector.tensor_copy(a, eslice)
                        else:
                            nc.vector.tensor_add(a, a, eslice)
                    # diff-4 left half, bottom-half rows: colsums feed the
                    # partner core's top-half rows (via the host)
                    if c0 == 4096 and m >= TPB // 2:
                        a3 = acc3[:]
                        eslice = e[:, 0:512]
                        if m == TPB // 2:
                            nc.vector.tensor_copy(a3, eslice)
                        else:
                            nc.vector.tensor_add(a3, a3, eslice)
                    # diag-block intra colsums: chunks c>m of this tile's
                    # trapezoid feed later tiles' row sums
                    if c0 < QCOLS:
                        for c in range(max(m + 1, c0 // 128), min((c0 + w) // 128, TPB)):
                            a2 = acc2[:, (c - 1) * 128:c * 128]
                            eslice = e[:, c * 128 - c0:(c + 1) * 128 - c0]
                            if m == 0:
                                nc.vector.tensor_copy(a2, eslice)
                            else:
                                nc.vector.tensor_add(a2, a2, eslice)
                nsl = 4 if m == 0 else 3
                nc.vector.reduce_sum(out=sv[:, m:m + 1],
                                     in_=parts[:, m * 4:m * 4 + nsl], axis=AX)

        # ---- tail: transpose-reduce column sums ----
        NK = CSBLKS * TPB
        NK2 = NK + TPB - 1
        with tc.tile_pool(name=f"tpsum{rep}", bufs=1, space="PSUM") as tpsum:
            cspT = tpsum.tile([128, NK2 + 4], f32, tag="cspT")
            for k in range(NK):
                # cspT[c, k] = sum_p acc[p, 128k + c]  (all output cols equal)
                nc.tensor.matmul(cspT[:, k:k + 1],
                                 acc[:, k * 128:(k + 1) * 128], ones[:, 0:1],
                                 start=True, stop=True)
            for c in range(1, TPB):
                nc.tensor.matmul(cspT[:, NK + c - 1:NK + c],
                                 acc2[:, (c - 1) * 128:c * 128], ones[:, 0:1],
                                 start=True, stop=True)
            for k in range(4):
                nc.tensor.matmul(cspT[:, NK2 + k:NK2 + k + 1],
                                 acc3[:, k * 128:(k + 1) * 128], ones[:, 0:1],
                                 start=True, stop=True)
            nc.vector.tensor_copy(csg[:], cspT[:])
        nc.sync.dma_start(out=s_out, in_=sv[:])
        nc.sync.dma_start(out=cs_out, in_=csg[:])


def _build_nc(repeats=1):
    import concourse.tile as tile
    import concourse.mybir as mybir
    from concourse import bacc

    f32 = mybir.dt.float32
    bf16 = mybir.dt.bfloat16
    nc = bacc.Bacc(
        "TRN2", target_bir_lowering=False, debug=False,
        enable_asserts=False, num_devices=NCORES,
    )
    nfT_h = nc.dram_tensor("nfT", [128, NCOLS], bf16, kind="ExternalInput")
    mk_h = nc.dram_tensor("masks", [128, 256], bf16, kind="ExternalInput")
    s_h = nc.dram_tensor("s_out", [128, TPB], f32, kind="ExternalOutput")
    c_h = nc.dram_tensor("cs_out", [128, CSOUT], f32, kind="ExternalOutput")

    with tile.TileContext(nc, trace_sim=False) as tc:
        for rep in range(repeats):
            _emit(tc, nfT_h.ap(), mk_h.ap(), s_h.ap(), c_h.ap(), rep=rep)
    nc.compile()
    return nc


def get_nc(repeats=1):
    key = ("nc", repeats)
    if key not in _CACHE:
        _CACHE[key] = _build_nc(repeats)
    return _CACHE[key]


def _host_prep(feats, label):
    """Normalize on host, build per-core rolled nfT (bf16) and host pos."""
    import ml_dtypes

    feats = np.asarray(feats, dtype=np.float32)
    label = np.asarray(label)
    norms = np.sqrt((feats.astype(np.float64) ** 2).sum(axis=1))
    nf = feats / np.maximum(norms, EPS)[:, None].astype(np.float32)
    pos_idx = np.argmax(label, axis=1)
    pos = np.einsum("ij,ij->i", nf.astype(np.float64), nf[pos_idx].astype(np.float64))
    nfT_full = np.ascontiguousarray(nf.T.astype(ml_dtypes.bfloat16))  # [128, N]
    nfT2 = np.concatenate([nfT_full, nfT_full[:, :NCOLS]], axis=1)
    masks = np.concatenate([np.eye(128), -MASK_SUB * np.eye(128)],
                           axis=1).astype(ml_dtypes.bfloat16)
    in_maps = []
    for c in range(NCORES):
        in_maps.append({
            "nfT": np.ascontiguousarray(nfT2[:, c * BLK:c * BLK + NCOLS]),
            "masks": masks,
        })
    return in_maps, pos


def make_in_maps(feats, label):
    in_maps, _ = _host_prep(feats, label)
    return in_maps


def finish(results, pos):
    """Host epilogue: assemble full row sums from direct row partials and
    symmetric column partials, then logsumexp and mean."""
    NK = CSBLKS * TPB
    NK2 = NK + TPB - 1
    S = np.zeros(N, dtype=np.float64)
    for x in range(NCORES):
        sv = results[x]["s_out"].astype(np.float64)       # [128, TPB]
        S[x * BLK:(x + 1) * BLK] += sv.T.reshape(-1)      # local rows in order
        csg = results[x]["cs_out"].astype(np.float64)     # [128, CSOUT]
        # cols 0..NK: colsums of local column 128k+c (k = 8*(q-1)+sub)
        cs = csg[:, :NK].T.reshape(CSBLKS, BLK)
        for k in range(1, CSBLKS + 1):
            tgt = ((x + k) % NCORES) * BLK                # rows of block x+k
            S[tgt:tgt + BLK] += cs[k - 1]
        # cols NK..NK2: diag-block lower-triangle sums for own rows 128..1023
        S[x * BLK + 128:(x + 1) * BLK] += csg[:, NK:NK2].T.reshape(-1)
        # cols NK2..: diff-4 bottom-left quadrant sums for partner rows 0..511
        tgt = ((x + 4) % NCORES) * BLK
        S[tgt:tgt + 512] += csg[:, NK2:NK2 + 4].T.reshape(-1)
    lse = np.log(S)
    loss = (lse - pos / TEMP).mean()
    return np.array(loss, dtype=np.float32)


def kernel(feats, label, _trace=False, _repeats=1):
    global LAST_RESULT
    from concourse.bass_utils import run_bass_kernel_spmd

    nc = get_nc(_repeats)
    in_maps, pos = _host_prep(feats, label)
    res = run_bass_kernel_spmd(nc, in_maps, list(range(NCORES)), trace=_trace)
    LAST_RESULT = res
    return finish(res.results, pos)


# revision 14
# speedup vs baseline: 1.1326x; 1.1023x over previous
"""Contrastive loss (SimCLR-style) on 8 Trainium2 NeuronCores.

Full inputs in, full output out.  Host pre-normalizes feats (f32), takes
the positive-pair cosines on the host (8192x128 dot products - trivial
next to the 256MB label argmax already done there), and ships each core
a rolled, transposed bf16 copy nfT = roll(nf).T of the 5120 columns the
core actually touches (1.25MB).  The device then only does the heavy
part: the N/8 x 5N/8 block of similarities, exp, and row/column sums.

Symmetry split, three levels (exp(cos/T) is symmetric):
 - inter-block: core x computes column blocks x..x+4 directly; row sums
   over blocks x+5..x+7 are recovered from column sums of (x, x+1..x+3).
 - diagonal block: M-tile m computes only cols >= m*128 (trapezoid); the
   lower triangle comes back as column sums of earlier tiles (acc2).
 - diff-4 block (computed by both x and x+4): each core does the left
   512 cols for its top-half rows plus full width for its bottom half;
   bottom-left-quadrant column sums are shipped to the partner (acc3).
Per M-tile the columns run as PSUM spans up to 2048 wide so the ACT exp
(the bottleneck engine - exp only exists there, (N+352)cyc @1.2GHz)
amortizes its per-instruction ramp; an exp-table warmup at t=0 hides
the ~2.7us table load under the input DMA.  The self-mask is a PE
accumulate-matmul (-30I)^T @ I so exp never waits on another engine.
Column sums accumulate on the otherwise-idle DVE in bf16; the final
partition reduction is 35 skinny stationary=acc matmuls that write a
transposed [128, 35] tile so the output DMA uses all 128 partitions
instead of a slow single-partition line.  Host: assemble S from row
partials + shipped column sums, logsumexp, mean over host-side pos.
"""

from contextlib import ExitStack

import numpy as np

N, D, NCORES = 8192, 128, 8
BLK = N // NCORES            # 1024 rows per core
TPB = BLK // 128             # 8 M-tiles (of 128 rows) per core
TEMP = 0.07
EPS = 1e-8
MASK_SUB = 30.0              # cos - 30 -> exp((cos-30)/T) == 0 in fp32
QCOLS = 1024                 # one column block
MMCHUNK = 512                # matmul moving-operand columns
NQ = 5                       # direct column blocks per core (cols 0..5119)
CSBLKS = 3                   # column-sum blocks (local col blocks 1..3)
NCOLS = NQ * QCOLS           # 5120 columns shipped per core
CSOUT = CSBLKS * TPB + TPB - 1 + 4   # shipped colsum chunks (inter|diag|diff4)

_CACHE = {}
LAST_RESULT = None


def _emit(tc, nfT_d, masks_d, s_out, cs_out, rep=0):
    import concourse.mybir as mybir

    nc = tc.nc
    f32 = mybir.dt.float32
    bf16 = mybir.dt.bfloat16
    AF = mybir.ActivationFunctionType
    AX = mybir.AxisListType.X

    with ExitStack() as ctx:
        singles = ctx.enter_context(tc.tile_pool(name=f"singles{rep}", bufs=1))

        nfT = singles.tile([128, NCOLS], bf16, tag="nfT")    # normalized X^T
        masks = singles.tile([128, 256], bf16, tag="masks")  # [ident | -30*ident]
        ones = singles.tile([128, 512], bf16, tag="ones")
        parts = singles.tile([128, TPB * 4], f32, tag="parts")
        sv = singles.tile([128, TPB], f32, tag="sv")
        acc = singles.tile([128, CSBLKS * QCOLS], bf16, tag="acc")
        acc2 = singles.tile([128, (TPB - 1) * 128], bf16, tag="acc2")
        acc3 = singles.tile([128, 512], bf16, tag="acc3")
        csg = singles.tile([128, CSBLKS * TPB + TPB - 1 + 4], f32, tag="csg")
        wact = singles.tile([128, 8], f32, tag="wact")

        # ---- ACT warmup, overlapped with the input DMA window ----
        # The first ACTIVATE pays the ~2.7us exp table load; trigger it at
        # t=0 on a tiny memset tile so the load hides under the nfT DMA.
        # (No PE warmup: its PSUM slot reuse makes the first real matmul
        # wait for every dummy one, which costs more than the cold clock.)
        nc.vector.memset(wact[:], 0.0)
        nc.scalar.activation(wact[:], wact[:], AF.Exp)
        nc.vector.memset(ones[:], 1.0)

        # ---- loads (chunk0 gates the first span; masks right behind) ----
        nc.sync.dma_start(out=nfT[:, 0:QCOLS], in_=nfT_d[:, 0:QCOLS])
        nc.sync.dma_start(out=masks[:], in_=masks_d)
        for a in range(QCOLS, NCOLS, QCOLS):
            nc.sync.dma_start(out=nfT[:, a:a + QCOLS], in_=nfT_d[:, a:a + QCOLS])

        # ---- similarity spans + exp row-sums + column sums ----
        # The diagonal block is symmetric, so M-tile m only computes cols
        # >= m*128 of it (trapezoid span); the missing lower-triangle row
        # contributions are recovered from colsums of earlier tiles' exp
        # (chunks accumulated into acc2) and folded into sv on-device.
        # m=0 runs 1024-wide spans so the first exp is gated by only two
        # matmuls on the freshly-DMAed chunk0, not four.
        SPANS0 = ((0, 1024, 0), (1024, 1024, 1), (2048, 2048, 2), (4096, 512, 3))
        with (
            tc.tile_pool(name=f"mpsum{rep}", bufs=2, space="PSUM") as mpsum,
            tc.tile_pool(name=f"escratch{rep}", bufs=3) as esp,
            tc.high_priority(),
        ):
            for m in range(TPB):
                lhsT = nfT[:, m * 128:(m + 1) * 128]
                if m == 0:
                    spans = SPANS0
                else:
                    w2 = 512 if m < TPB // 2 else 1024
                    spans = ((m * 128, 2048 - m * 128, 0), (2048, 2048, 1),
                             (4096, w2, 2))
                for c0, w, slot in spans:
                    pt = mpsum.tile([128, 2048], f32, tag="mp")
                    b = c0
                    while b < c0 + w:
                        cw = min(MMCHUNK, c0 + w - b)
                        nc.tensor.matmul(
                            pt[:, b - c0:b - c0 + cw],
                            lhsT, nfT[:, b:b + cw],
                            start=True, stop=True,
                        )
                        b += cw
                    if c0 == m * 128:
                        # self column of local row m*128+p is m*128+p (rolled
                        # input): accumulate (-30I)^T @ I onto the diagonal
                        # sub-block on the PE so exp never waits on another
                        # engine.
                        nc.tensor.matmul(
                            pt[:, 0:128],
                            masks[:, 128:256], masks[:, 0:128],
                            start=False, stop=True, skip_group_check=True,
                        )
                    e = esp.tile([128, 2048], bf16, tag="e")
                    nc.scalar.activation(
                        e[:, :w], pt[:, :w], AF.Exp, scale=1.0 / TEMP,
                        accum_out=parts[:, m * 4 + slot:m * 4 + slot + 1],
                    )
                    # column-sum accumulation (cols 1024..4095) on DVE
                    lo = max(c0, QCOLS)
                    hi = min(c0 + w, (CSBLKS + 1) * QCOLS)
                    if lo < hi:
                        a = acc[:, lo - QCOLS:hi - QCOLS]
                        eslice = e[:, lo - c0:hi - c0]
                        if m == 0:
                            nc.vector.tensor_copy(a, eslice)
                        else:
                            nc.vector.tensor_add(a, a, eslice)
                    # diff-4 left half, bottom-half rows: colsums feed the
                    # partner core's top-half rows (via the host)
                    if c0 == 4096 and m >= TPB // 2:
                        a3 = acc3[:]
                        eslice = e[:, 0:512]
                        if m == TPB // 2:
                            nc.vector.tensor_copy(a3, eslice)
                        else:
                            nc.vector.tensor_add(a3, a3, eslice)
                    # diag-block intra colsums: chunks c>m of this tile's
                    # trapezoid feed later tiles' row sums.  Chunks m+1..7
                    # are contiguous in both acc2 and e: one wide op per m.
                    if c0 == m * 128 and m < TPB - 1:
                        a2 = acc2[:, m * 128:(TPB - 1) * 128]
                        eslice = e[:, 128:(TPB - m) * 128]
                        if m == 0:
                            nc.vector.tensor_copy(a2, eslice)
                        else:
                            nc.vector.tensor_add(a2, a2, eslice)
                nsl = 4 if m == 0 else 3
                nc.vector.reduce_sum(out=sv[:, m:m + 1],
                                     in_=parts[:, m * 4:m * 4 + nsl], axis=AX)

        # ---- tail: transpose-reduce column sums ----
        NK = CSBLKS * TPB
        NK2 = NK + TPB - 1
        with tc.tile_pool(name=f"tpsum{rep}", bufs=1, space="PSUM") as tpsum:
            cspT = tpsum.tile([128, NK2 + 4], f32, tag="cspT")
            for k in range(NK):
                # cspT[c, k] = sum_p acc[p, 128k + c]  (all output cols equal)
                nc.tensor.matmul(cspT[:, k:k + 1],
                                 acc[:, k * 128:(k + 1) * 128], ones[:, 0:1],
                                 start=True, stop=True)
            for c in range(1, TPB):
                nc.tensor.matmul(cspT[:, NK + c - 1:NK + c],
                                 acc2[:, (c - 1) * 128:c * 128], ones[:, 0:1],
                                 start=True, stop=True)
            for k in range(4):
                nc.tensor.matmul(cspT[:, NK2 + k:NK2 + k + 1],
                                 acc3[:, k * 128:(k + 1) * 128], ones[:, 0:1],
                                 start=True, stop=True)
            nc.vector.tensor_copy(csg[:], cspT[:])
        nc.sync.dma_start(out=s_out, in_=sv[:])
        nc.sync.dma_start(out=cs_out, in_=csg[:])


def _build_nc(repeats=1):
    import concourse.tile as tile
    import concourse.mybir as mybir
    from concourse import bacc

    f32 = mybir.dt.float32
    bf16 = mybir.dt.bfloat16
    nc = bacc.Bacc(
        "TRN2", target_bir_lowering=False, debug=False,
        enable_asserts=False, num_devices=NCORES,
    )
    nfT_h = nc.dram_tensor("nfT", [128, NCOLS], bf16, kind="ExternalInput")
    mk_h = nc.dram_tensor("masks", [128, 256], bf16, kind="ExternalInput")
    s_h = nc.dram_tensor("s_out", [128, TPB], f32, kind="ExternalOutput")
    c_h = nc.dram_tensor("cs_out", [128, CSOUT], f32, kind="ExternalOutput")

    with tile.TileContext(nc, trace_sim=False) as tc:
        for rep in range(repeats):
            _emit(tc, nfT_h.ap(), mk_h.ap(), s_h.ap(), c_h.ap(), rep=rep)
    nc.compile()
    return nc


def get_nc(repeats=1):
    key = ("nc", repeats)
    if key not in _CACHE:
        _CACHE[key] = _build_nc(repeats)
    return _CACHE[key]


def _host_prep(feats, label):
    """Normalize on host, build per-core rolled nfT (bf16) and host pos."""
    import ml_dtypes

    feats = np.asarray(feats, dtype=np.float32)
    label = np.asarray(label)
    norms = np.sqrt((feats.astype(np.float64) ** 2).sum(axis=1))
    nf = feats / np.maximum(norms, EPS)[:, None].astype(np.float32)
    pos_idx = np.argmax(label, axis=1)
    pos = np.einsum("ij,ij->i", nf.astype(np.float64), nf[pos_idx].astype(np.float64))
    nfT_full = np.ascontiguousarray(nf.T.astype(ml_dtypes.bfloat16))  # [128, N]
    nfT2 = np.concatenate([nfT_full, nfT_full[:, :NCOLS]], axis=1)
    masks = np.concatenate([np.eye(128), -MASK_SUB * np.eye(128)],
                           axis=1).astype(ml_dtypes.bfloat16)
    in_maps = []
    for c in range(NCORES):
        in_maps.append({
            "nfT": np.ascontiguousarray(nfT2[:, c * BLK:c * BLK + NCOLS]),
            "masks": masks,
        })
    return in_maps, pos


def make_in_maps(feats, label):
    in_maps, _ = _host_prep(feats, label)
    return in_maps


def finish(results, pos):
    """Host epilogue: assemble full row sums from direct row partials and
    symmetric column partials, then logsumexp and mean."""
    NK = CSBLKS * TPB
    NK2 = NK + TPB - 1
    S = np.zeros(N, dtype=np.float64)
    for x in range(NCORES):
        sv = results[x]["s_out"].astype(np.float64)       # [128, TPB]
        S[x * BLK:(x + 1) * BLK] += sv.T.reshape(-1)      # local rows in order
        csg = results[x]["cs_out"].astype(np.float64)     # [128, CSOUT]
        # cols 0..NK: colsums of local column 128k+c (k = 8*(q-1)+sub)
        cs = csg[:, :NK].T.reshape(CSBLKS, BLK)
        for k in range(1, CSBLKS + 1):
            tgt = ((x + k) % NCORES) * BLK                # rows of block x+k
            S[tgt:tgt + BLK] += cs[k - 1]
        # cols NK..NK2: diag-block lower-triangle sums for own rows 128..1023
        S[x * BLK + 128:(x + 1) * BLK] += csg[:, NK:NK2].T.reshape(-1)
        # cols NK2..: diff-4 bottom-left quadrant sums for partner rows 0..511
        tgt = ((x + 4) % NCORES) * BLK
        S[tgt:tgt + 512] += csg[:, NK2:NK2 + 4].T.reshape(-1)
    lse = np.log(S)
    loss = (lse - pos / TEMP).mean()
    return np.array(loss, dtype=np.float32)


def kernel(feats, label, _trace=False, _repeats=1):
    global LAST_RESULT
    from concourse.bass_utils import run_bass_kernel_spmd

    nc = get_nc(_repeats)
    in_maps, pos = _host_prep(feats, label)
    res = run_bass_kernel_spmd(nc, in_maps, list(range(NCORES)), trace=_trace)
    LAST_RESULT = res
    return finish(res.results, pos)
